# revision 6
# baseline (speedup 1.0000x reference)
"""Multi-head latent attention (MLA) forward pass on 8 Trainium2 NeuronCores.

Sharding: 2 (batch) x 4 (head-group) grid. Core c handles batch b = c // 4
and heads 4*(c % 4) .. 4*(c % 4) + 3.

v2 design (vs. baseline): all layout transposes moved off the PE onto the
DMA xbar-transpose engine; attention computed in q-major score form so the
softmax denominator comes free from the ACT accumulator (no ones-matmuls);
PV runs k-outer over groups of 4 q-tiles with wide (<=512) moving operands;
kv_up is software-pipelined into the GEMM-A s-tile loop; out-projection is
emitted per q-group right after attention so the PE queue never drains.

Per core:
  s-loop (16 s-tiles of 128):
    A = x_tile @ [Wq | Wkrope | Wkv_down]  (PSUM, 1280 cols)
    batched rms-norm stats via DVE square + grouped reduce, gains folded
    rope on q/k_rope in place; Q / c_kv / [k_nope|k_rope] transposed to
    head-dim-major via DMA xbar (3 transposes per s-tile, zero PE work)
    kv_up GEMM (lagged one s-tile), k_nope norm, V evict
  attention (per q-group g of 4 q-tiles, per head):
    S[q,k] staircase chunks (<=512 k) = QT_tile.T @ KT  (exact causal)
    additive -1e9 mask on the diagonal tile, exp on ACT with accum_out
    giving l[q]; P *= 1/l on DVE; P^T via DMA xbar into a per-group tile
    PV: y^T[d,q] accumulated k-outer with V stationary, moving P^T spans
    all 4 q-tiles (N up to 512)
  proj per s-tile of the group: out[s,:] = sum_h yT_h.T @ Wp_h
Host sums the 4 partials per batch element.
"""

import sys

for _p in ("/opt/trn_rl_repo",):
    if _p not in sys.path:
        sys.path.insert(0, _p)

import math
from contextlib import ExitStack

import ml_dtypes
import numpy as np

import concourse.bass as bass
import concourse.mybir as mybir
import concourse.tile as tile
from concourse import bacc
from concourse.bass_utils import run_bass_kernel_spmd

F32 = mybir.dt.float32
BF16 = mybir.dt.bfloat16
BF = ml_dtypes.bfloat16

B, S, D = 2, 2048, 2048
H = 16
HD = 128           # head dim
ROPE = 64
NOPE = 64
LAT = 512
EPS = 1e-6
ROPE_BASE = 10000.0

H_LOC = 4          # heads per core
N_CORES = 8
DLOC = H_LOC * HD  # 512, per-core proj contraction size

ST_N = S // 128    # 16 s-tiles
KT_N = D // 128    # 16 k-tiles for GEMM-A

A_QW = H_LOC * HD             # 512  q columns in A
A_RW = H_LOC * ROPE           # 256  k_rope columns in A
A_W = A_QW + A_RW + LAT       # 1280 total A columns
KV_W = H_LOC * NOPE + H_LOC * HD   # 768 kv columns

X8_CHUNK = 256                # s-columns of x^T per streamed chunk
X8_N = S // X8_CHUNK          # 8 chunks
ST_PER_CHUNK = X8_CHUNK // 128  # 2

MULT = mybir.AluOpType.mult
ADD = mybir.AluOpType.add
SUB = mybir.AluOpType.subtract
EXPF = mybir.ActivationFunctionType.Exp
SQRTF = mybir.ActivationFunctionType.Sqrt
SQF = mybir.ActivationFunctionType.Square
AXX = mybir.AxisListType.X
AXXY = mybir.AxisListType.XY

_PROGRAM_CACHE = {}


def _build_program():
    nc = bacc.Bacc(None, target_bir_lowering=False, debug=True)

    # ---- DRAM I/O ----
    xT8 = nc.dram_tensor("xT8", [X8_N, D, X8_CHUNK], BF16, kind="ExternalInput")
    w_a = nc.dram_tensor("w_a", [D, A_W], BF16, kind="ExternalInput")
    w_up = nc.dram_tensor("w_up", [LAT, KV_W], BF16, kind="ExternalInput")
    w_p = nc.dram_tensor("w_p", [DLOC, D], BF16, kind="ExternalInput")
    cos4 = nc.dram_tensor("cos4", [S, H_LOC, ROPE // 2], BF16, kind="ExternalInput")
    sin4 = nc.dram_tensor("sin4", [S, H_LOC, ROPE // 2], BF16, kind="ExternalInput")
    maskadd = nc.dram_tensor("maskadd", [128, 128], F32, kind="ExternalInput")
    gain13 = nc.dram_tensor("gain13", [128, 13], F32, kind="ExternalInput")
    out = nc.dram_tensor("out", [S, D], F32, kind="ExternalOutput")

    inv_sqrt_hd = 1.0 / math.sqrt(HD)

    with tile.TileContext(nc) as tc, ExitStack() as top:
        const = top.enter_context(tc.tile_pool(name="const", bufs=1))
        big = top.enter_context(tc.tile_pool(name="big", bufs=1))

        # --- persistent activations (head-dim-major) ---
        QT = big.tile([128, H_LOC, S], BF16)   # [d, h, q]
        KT = big.tile([128, H_LOC, S], BF16)   # [d, h, k] (0:64 nope, 64:128 rope)
        V = big.tile([128, ST_N, H_LOC * HD], BF16)  # [s%128, s//128, d_loc]
        yT = big.tile([128, H_LOC, S], BF16)   # [d, h, q]

        mask_sb = const.tile([128, 128], F32)
        gain_sb = const.tile([128, 13], F32)
        eps_sb = const.tile([128, 1], F32)
        wp_sb = const.tile([128, H_LOC, D], BF16)  # loaded late

        # ===================== phase S: s-tile loop =====================
        sphase = ExitStack()
        wpool = sphase.enter_context(tc.tile_pool(name="wpool", bufs=1))
        wa_sb = wpool.tile([128, KT_N, A_W], BF16)
        wa_r = w_a[:].rearrange("(k p) n -> p k n", p=128)
        nc.sync.dma_start(out=wa_sb[:, 0, :], in_=wa_r[:, 0, :])

        x8p = sphase.enter_context(tc.tile_pool(name="x8p", bufs=2))
        xq0 = x8p.tile([128, KT_N, X8_CHUNK], BF16, tag="x8")
        nc.sync.dma_start(out=xq0[:], in_=xT8[0].rearrange("(k p) s -> p k s", p=128))
        for kt in range(1, KT_N):
            nc.sync.dma_start(out=wa_sb[:, kt, :], in_=wa_r[:, kt, :])

        wup_sb = wpool.tile([128, LAT // 128, KV_W], BF16)
        nc.sync.dma_start(out=wup_sb[:], in_=w_up[:].rearrange("(k p) n -> p k n", p=128))
        cos_sb = wpool.tile([128, ST_N, H_LOC, ROPE // 2], BF16)
        nc.sync.dma_start(out=cos_sb[:], in_=cos4[:].rearrange("(t p) h f -> p t h f", p=128))
        sin_sb = wpool.tile([128, ST_N, H_LOC, ROPE // 2], BF16)
        nc.sync.dma_start(out=sin_sb[:], in_=sin4[:].rearrange("(t p) h f -> p t h f", p=128))
        nc.sync.dma_start(out=gain_sb[:], in_=gain13[:])
        nc.sync.dma_start(out=mask_sb[:], in_=maskadd[:])
        nc.vector.memset(eps_sb[:], EPS)

        ckvT = big.tile([128, LAT // 128, S], BF16)  # [lat, lt, s]
        psA = sphase.enter_context(tc.tile_pool(name="psA", bufs=2, space="PSUM"))
        scr = sphase.enter_context(tc.tile_pool(name="scr", bufs=3))
        jnk = sphase.enter_context(tc.tile_pool(name="jnk", bufs=2))

        def rsqrt_act(dst, src, n):
            """dst = 1/sqrt(src/n + eps): ACT Sqrt then fast DVE reciprocal."""
            nc.scalar.activation(dst, src, SQRTF, scale=1.0 / n, bias=eps_sb[:])
            nc.vector.reciprocal_approx_fast(out=dst, in_=dst)

        def emit_gemm_a(ST):
            e, st2 = divmod(ST, ST_PER_CHUNK)
            if st2 == 0:
                if e == 0:
                    xq = xq0
                else:
                    xq = x8p.tile([128, KT_N, X8_CHUNK], BF16, tag="x8")
                    nc.sync.dma_start(
                        out=xq[:], in_=xT8[e].rearrange("(k p) s -> p k s", p=128))
                emit_gemm_a.xq = xq
            xq = emit_gemm_a.xq
            aps = psA.tile([128, A_W], F32, tag="A")
            for kt in range(KT_N):
                lhs = xq[:, kt, st2 * 128:(st2 + 1) * 128]
                for c0, c1 in ((0, 512), (512, 1024), (1024, 1280)):
                    nc.tensor.matmul(
                        aps[:, c0:c1], lhs, wa_sb[:, kt, c0:c1],
                        start=(kt == 0), stop=(kt == KT_N - 1))
            return aps

        def emit_norms(ST, aps):
            s0 = ST * 128
            junk = jnk.tile([128, A_W], BF16, tag="junk")
            nc.scalar.activation(junk[:], aps[:], SQF)
            rs13 = scr.tile([128, 13], F32, tag="rs13")
            nc.vector.tensor_reduce(
                rs13[:, 0:12],
                junk[:, 0:768].rearrange("p (g c) -> p g c", c=64),
                AXX, ADD)
            nc.vector.tensor_reduce(
                rs13[:, 12:13],
                junk[:, 768:1280].rearrange("p (g c) -> p g c", c=64),
                AXXY, ADD)
            rsqrt_act(rs13[:, 0:12], rs13[:, 0:12], 64)
            rsqrt_act(rs13[:, 12:13], rs13[:, 12:13], LAT)
            nc.vector.tensor_tensor(rs13[:], rs13[:], gain_sb[:], MULT)

            # ---- apply norms ----
            nrmq = scr.tile([128, A_QW], BF16, tag="nrmq")
            nc.vector.tensor_tensor(
                nrmq[:].rearrange("p (g c) -> p g c", c=64),
                aps[:, 0:512].rearrange("p (g c) -> p g c", c=64),
                rs13[:, 0:8].to_broadcast([128, 8, 64]), MULT)
            kcomb = scr.tile([128, H_LOC, HD], BF16, tag="kcomb")
            nc.vector.tensor_tensor(
                kcomb[:, :, NOPE:HD].rearrange("p h c -> p h c"),
                aps[:, 512:768].rearrange("p (h c) -> p h c", c=64),
                rs13[:, 8:12].to_broadcast([128, 4, 64]), MULT)
            cv = scr.tile([128, LAT], BF16, tag="cv")
            nc.vector.tensor_scalar(
                cv[:], aps[:, 768:1280], rs13[:, 12:13], None, MULT)

            # ---- rope (in place; temps carry all products first) ----
            RH = ROPE // 2
            cos_ap = cos_sb[:, ST]
            sin_ap = sin_sb[:, ST]
            nq = nrmq[:].rearrange("p (h t c) -> p h t c", t=2, c=64)

            def rope_inplace(x1, x2):
                t1 = scr.tile([128, H_LOC, RH], F32, tag="t1")
                t2 = scr.tile([128, H_LOC, RH], F32, tag="t2")
                t3 = scr.tile([128, H_LOC, RH], F32, tag="t3")
                t4 = scr.tile([128, H_LOC, RH], F32, tag="t4")
                nc.vector.tensor_tensor(t1[:], x1, cos_ap, MULT)
                nc.vector.tensor_tensor(t2[:], x2, sin_ap, MULT)
                nc.vector.tensor_tensor(t3[:], x2, cos_ap, MULT)
                nc.vector.tensor_tensor(t4[:], x1, sin_ap, MULT)
                nc.vector.tensor_tensor(x1, t1[:], t2[:], ADD)
                nc.vector.tensor_tensor(x2, t3[:], t4[:], SUB)

            rope_inplace(nq[:, :, 1, 0:RH], nq[:, :, 1, RH:ROPE])
            rope_inplace(kcomb[:, :, NOPE:NOPE + RH], kcomb[:, :, NOPE + RH:HD])

            # ---- DMA xbar transposes into head-dim-major tiles ----
            nc.sync.dma_start(
                out=QT[:, :, s0:s0 + 128], in_=nrmq[:], transpose=True)
            nc.sync.dma_start(
                out=ckvT[:, :, s0:s0 + 128], in_=cv[:], transpose=True)
            return kcomb

        def emit_kv_up(ST, kcomb):
            s0 = ST * 128
            kvps = psA.tile([128, KV_W], F32, tag="A")
            for lt in range(LAT // 128):
                lhs = ckvT[:, lt, s0:s0 + 128]
                for c0, c1 in ((0, 512), (512, 768)):
                    nc.tensor.matmul(
                        kvps[:, c0:c1], lhs, wup_sb[:, lt, c0:c1],
                        start=(lt == 0), stop=(lt == LAT // 128 - 1))
            junkk = jnk.tile([128, H_LOC * NOPE], BF16, tag="junkk")
            nc.scalar.activation(junkk[:], kvps[:, 0:256], SQF)
            rsk = scr.tile([128, 4], F32, tag="rsk")
            nc.vector.tensor_reduce(
                rsk[:], junkk[:].rearrange("p (g c) -> p g c", c=64),
                AXX, ADD)
            rsqrt_act(rsk[:], rsk[:], 64)
            nc.vector.tensor_tensor(
                kcomb[:, :, 0:NOPE],
                kvps[:, 0:256].rearrange("p (g c) -> p g c", c=64),
                rsk[:].to_broadcast([128, 4, 64]), MULT)
            nc.scalar.copy(V[:, ST, :], kvps[:, H_LOC * NOPE:KV_W])
            nc.sync.dma_start(
                out=KT[:, :, s0:s0 + 128], in_=kcomb[:], transpose=True)

        prev = None  # (ST, aps->kcomb chain state)
        for ST in range(ST_N):
            aps = emit_gemm_a(ST)
            if prev is not None:
                emit_kv_up(prev[0], prev[1])
            kcomb = emit_norms(ST, aps)
            prev = (ST, kcomb)
        emit_kv_up(prev[0], prev[1])

        sphase.close()

        # late load for attention/proj phase
        nc.sync.dma_start(out=wp_sb[:], in_=w_p[:].rearrange("(k p) n -> p k n", p=128))

        # =========== phase T: attention + out projection per q-group ========
        # Software-pipelined emission: PV for pair p is emitted after the
        # scores of pair p+1, and the out-projection of group g after the
        # scores of the pair following PV(g, 3) — so the in-order PE queue
        # always has independent matmuls between a chain's producer and
        # consumer (exp -> 1/l scale -> DMA transpose latency is hidden).
        with (
            tc.tile_pool(name="pS", bufs=2, space="PSUM") as pS,
            tc.tile_pool(name="pY", bufs=2, space="PSUM") as pY,
            tc.tile_pool(name="pO", bufs=4, space="PSUM") as pO,
            tc.tile_pool(name="pq", bufs=3) as pq,
            tc.tile_pool(name="pt", bufs=2) as pt,
            tc.tile_pool(name="psc", bufs=8) as psc,
            tc.tile_pool(name="po", bufs=4) as po,
        ):
            def emit_scores(g, h):
                PTg = pt.tile([128, ST_N, 4, 128], BF16, tag="PT")
                for qi in range(4):
                    i = 4 * g + qi
                    nkt = i + 1
                    Pq = pq.tile([128, S], BF16, tag="Pq")
                    nchunks = (nkt + 3) // 4
                    lacc = psc.tile([128, 4], F32, tag="lacc")
                    for c in range(nchunks):
                        kt0 = 4 * c
                        kt1 = min(kt0 + 4, nkt)
                        N = (kt1 - kt0) * 128
                        sps = pS.tile([128, 512], F32, tag="S")
                        nc.tensor.matmul(
                            sps[:, 0:N],
                            QT[:, h, i * 128:(i + 1) * 128],
                            KT[:, h, kt0 * 128:kt1 * 128],
                            start=True, stop=True)
                        if kt1 == nkt:  # chunk holds the diagonal tile
                            nc.vector.tensor_tensor(
                                sps[:, N - 128:N], sps[:, N - 128:N],
                                mask_sb[:], ADD)
                        nc.scalar.activation(
                            Pq[:, kt0 * 128:kt1 * 128], sps[:, 0:N],
                            EXPF, scale=inv_sqrt_hd,
                            accum_out=lacc[:, c:c + 1])
                    linv = psc.tile([128, 1], F32, tag="linv")
                    if nchunks > 1:
                        nc.vector.tensor_reduce(
                            linv[:], lacc[:, 0:nchunks], AXX, ADD)
                        nc.vector.reciprocal_approx_fast(
                            out=linv[:], in_=linv[:])
                    else:
                        nc.vector.reciprocal_approx_fast(
                            out=linv[:], in_=lacc[:, 0:1])
                    nc.vector.tensor_scalar(
                        Pq[:, 0:nkt * 128], Pq[:, 0:nkt * 128],
                        linv[:], None, MULT)
                    nc.sync.dma_start(
                        out=PTg[:, 0:nkt, qi, :], in_=Pq[:, 0:nkt * 128],
                        transpose=True)
                return PTg

            def emit_pv(g, h, PTg):
                yps = pY.tile([128, 4, 128], F32, tag="Y")
                for kt in range(4 * g + 4):
                    lo = max(kt - 4 * g, 0)
                    nc.tensor.matmul(
                        yps[:, lo:4, :],
                        V[:, kt, h * HD:(h + 1) * HD],
                        PTg[:, kt, lo:4, :],
                        start=(kt == 0), stop=(kt == 4 * g + 3),
                        skip_group_check=True)
                nc.vector.tensor_copy(
                    yT[:, h, g * 512:(g + 1) * 512],
                    yps[:].rearrange("p a b -> p (a b)"))

            def emit_proj(g):
                for t in range(4 * g, 4 * g + 4):
                    s0 = t * 128
                    for nb in range(D // 512):
                        ot = pO.tile([128, 512], F32, tag="O")
                        for h in range(H_LOC):
                            nc.tensor.matmul(
                                ot[:], yT[:, h, s0:s0 + 128],
                                wp_sb[:, h, nb * 512:(nb + 1) * 512],
                                start=(h == 0), stop=(h == H_LOC - 1))
                        osb = po.tile([128, 512], F32, tag="osb")
                        nc.vector.tensor_copy(osb[:], ot[:])
                        nc.sync.dma_start(
                            out=out[s0:s0 + 128, nb * 512:(nb + 1) * 512],
                            in_=osb[:])

            pairs = [(g, h) for g in range(ST_N // 4) for h in range(H_LOC)]
            pend_pv = None    # (g, h, PTg) with scores emitted, PV not yet
            pend_proj = None  # group whose PV is done but proj not emitted
            for g, h in pairs:
                PTg = emit_scores(g, h)
                if pend_pv is not None:
                    pg, ph, pt_tile = pend_pv
                    emit_pv(pg, ph, pt_tile)
                    if pend_proj is not None:
                        emit_proj(pend_proj)
                        pend_proj = None
                    if ph == H_LOC - 1:
                        pend_proj = pg
                pend_pv = (g, h, PTg)
            pg, ph, pt_tile = pend_pv
            emit_pv(pg, ph, pt_tile)
            if pend_proj is not None:
                emit_proj(pend_proj)
            emit_proj(pg)
    nc.compile()
    return nc


def _prep_inputs(x, w_q_krope, w_kv_down, w_kv_up, w_proj, q_gain):
    """Build the 8 per-core input maps (host-side sharding)."""
    inv_freq = ROPE_BASE ** (-np.arange(0, ROPE, 2, dtype=np.float32) / ROPE)
    t = np.arange(S, dtype=np.float32)
    freqs = np.outer(t, inv_freq)                      # (S, 32)
    cos4 = np.ascontiguousarray(np.broadcast_to(
        np.cos(freqs)[:, None, :], (S, H_LOC, ROPE // 2))).astype(BF)
    sin4 = np.ascontiguousarray(np.broadcast_to(
        np.sin(freqs)[:, None, :], (S, H_LOC, ROPE // 2))).astype(BF)

    qq = np.arange(128)[:, None]
    kk = np.arange(128)[None, :]
    maskadd = np.where(kk <= qq, 0.0, -1e9).astype(np.float32)  # [128q, 128k]

    # x^T per batch, chunked: [X8_N, D, X8_CHUNK]
    xT_chunks = []
    for b in range(B):
        xT = np.ascontiguousarray(x[b].T).astype(BF)   # [D, S]
        xT_chunks.append(np.ascontiguousarray(
            xT.reshape(D, X8_N, X8_CHUNK).transpose(1, 0, 2)))

    in_maps = []
    for c in range(N_CORES):
        b = c // H_LOC
        hg = c % H_LOC
        heads = [hg * H_LOC + i for i in range(H_LOC)]
        w_a = np.concatenate(
            [w_q_krope[:, h * HD:(h + 1) * HD] for h in heads]
            + [w_q_krope[:, D + h * ROPE:D + (h + 1) * ROPE] for h in heads]
            + [w_kv_down], axis=1).astype(BF)           # [D, 1280]
        w_up = np.concatenate(
            [w_kv_up[:, h * NOPE:(h + 1) * NOPE] for h in heads]
            + [w_kv_up[:, NOPE * H + h * HD:NOPE * H + (h + 1) * HD]
               for h in heads], axis=1).astype(BF)      # [LAT, 768]
        w_p = w_proj[hg * DLOC:(hg + 1) * DLOC, :].astype(BF)   # [512, D]
        g = q_gain[heads].astype(np.float32)
        g13 = np.concatenate([np.repeat(g, 2), np.ones(5, np.float32)])
        gain13 = np.ascontiguousarray(
            np.broadcast_to(g13[None, :], (128, 13))).astype(np.float32)
        in_maps.append({
            "xT8": xT_chunks[b],
            "w_a": np.ascontiguousarray(w_a),
            "w_up": np.ascontiguousarray(w_up),
            "w_p": np.ascontiguousarray(w_p),
            "cos4": cos4, "sin4": sin4, "maskadd": maskadd,
            "gain13": gain13,
        })
    return in_maps


def kernel(x, w_q_krope, w_kv_down, w_kv_up, w_proj, q_gain, **_unused):
    x = np.asarray(x, dtype=np.float32)
    w_q_krope = np.asarray(w_q_krope, dtype=np.float32)
    w_kv_down = np.asarray(w_kv_down, dtype=np.float32)
    w_kv_up = np.asarray(w_kv_up, dtype=np.float32)
    w_proj = np.asarray(w_proj, dtype=np.float32)
    q_gain = np.asarray(q_gain, dtype=np.float32)

    if "nc" not in _PROGRAM_CACHE:
        _PROGRAM_CACHE["nc"] = _build_program()
    nc = _PROGRAM_CACHE["nc"]

    in_maps = _prep_inputs(x, w_q_krope, w_kv_down, w_kv_up, w_proj, q_gain)
    res = run_bass_kernel_spmd(nc, in_maps, list(range(N_CORES)))

    out = np.zeros((B, S, D), dtype=np.float32)
    for c in range(N_CORES):
        out[c // H_LOC] += res.results[c]["out"]
    return out
